# revision 15
# baseline (speedup 1.0000x reference)
"""Trainium2 Bass kernel for nn_ODEFunc (gnn_message_passing, 8 cores).

Strategy:
  - Batch-parallel branches: core b computes batch b's diff+adv gconv
    branches (all 9 support matrices stream through the PE as fp16).
  - Transposed-mat layout [feature, node]; PE transposes flip layouts for
    the Chebyshev recurrence. Two-pass structure per layer (all x1
    matmuls, then per-support transpose+x2) keeps the PE dense.
  - L2 weights/biases are negated on the host so tanh emits the final
    grad sign for free; the diff branch's extra 0.1 factor is one DVE op.
  - Grads stay feature-major [16, 512] fp16 -> AllGather -> agout.
  - W_f GEMM contraction is enumerated k = f*512 + n so the gathered
    grads DMA straight into the stationary layout gt_all[128, KT, 40]
    (two strided DMAs, no transposes, no staging); wt is host-permuted
    to the same k order. psum[40, 1024]: X_diff rows 0-7, X_adv 32-39;
    cols 8-31 are memset-zero garbage lanes.
  - W_f shard (fp16, 16 MB) is split into 4 DMAs on 4 engine queues so
    no single queue serializes behind it.
  - Gated fusion (sigmoid) on-chip; core c returns out[:, c*1024:...].

Mat slot bases: matmul operands must start at partition 0/32/64 (other
engines also allow 96). L1 packs 4 16-row mats per 128-row K-tile; x1
mats (which feed PE transposes) go to bases 0/32, x0/x2 to 64/96, with
the L1 weight rows permuted on the host to match. L2 mats are 64-row:
x1 at base 64, x0/x2 at 0, all legal.
"""

import sys

sys.path.insert(0, "/opt/trn_rl_repo")

import numpy as np

import concourse.bass as bass
import concourse.mybir as mybir
from concourse import masks
from concourse.bass_utils import run_bass_kernel_spmd
from concourse.tile import TileContext
from concourse.vector_clock import ScopedClock

N = 512          # nodes
FL = 16          # latent
U = 64           # units
B = 8            # batch
HID = N * FL     # 8192
COEFF = 0.1
NCORES = 8
JS = HID // NCORES  # 1024 output columns per core
KT = HID // 128     # 64 contraction tiles for the W_f GEMM

f16 = mybir.dt.float16
f32 = mybir.dt.float32
AF = mybir.ActivationFunctionType
ALU = mybir.AluOpType


# L1 within-tile base for mat j (16-row mats in 128-row tiles of 4):
# x1 mats (odd j) at 0/32 so they are legal PE-transpose inputs.
def _l1_base(j):
    return {1: 0, 3: 32, 0: 64, 2: 96}[j % 4]


# smalls_f16 packed free-dim offsets (elements)
_OFF_X0M = 0          # [128, 4*16]
_OFF_WA1 = 64         # [128, 5*64]
_OFF_WD1 = 384        # [80(->128), 64]
_OFF_WA2 = 448        # [128, 9*16]
_OFF_WD2 = 592        # [128, 2*16]
_OFF_BF = 624         # [1, 1024]
_OFF_X0T = 1648       # [16, 512]
_SM16 = 2160


class PatchedTileContext(TileContext):
    """Tail drain with at most one sem wait per instruction.

    The walrus build here rejects Drain instructions carrying >2 sync
    waits ("Too many sync wait commands"). Spread the global-clock waits
    over individual SP nops ahead of the drain.
    """

    def _drain_and_barrier(self, tick_clock, wait_clock):
        nc = self.nc
        probe = nc.sync.nop(nofuse=True)
        wait_clock.add_sem_waits(
            probe.ins, ScopedClock({None: tick_clock.global_clock})
        )
        si = probe.ins.sync_info
        ws = list(si.on_wait) if si is not None else []
        if len(ws) > 1:
            probe.ins.sync_info = mybir.SyncInfo(
                on_wait=ws[:1], on_update=list(si.on_update)
            )
            for w in ws[1:]:
                n2 = nc.sync.nop(nofuse=True)
                n2.ins.sync_info = mybir.SyncInfo(on_wait=[w], on_update=[])
        nc.sync.drain()
        nc.all_engine_barrier()
        popped = nc._tile_sem_poison_stack.pop()
        assert popped is self._sem_poison
        nc.clear_and_free_semaphores(list(self.sems.allocated().values()))
        nc.all_engine_barrier()


_WAIT_LIMIT = 1


def _split_excess_waits(nc: bass.Bass) -> None:
    """Move sync waits beyond _WAIT_LIMIT onto same-engine NOPs inserted
    just before the carrying instruction (this walrus build has tiny
    setupSyncWait budgets for DMA/collective/drain instruction formats)."""
    for fn in nc.m.functions:
        for bb in fn.blocks:
            insts = bb.instructions
            i = 0
            while i < len(insts):
                inst = insts[i]
                si = inst.sync_info
                ws = list(si.on_wait) if si is not None and si.on_wait else []
                if len(ws) > _WAIT_LIMIT and type(inst).__name__ != "InstNoOp":
                    keep = ws[:_WAIT_LIMIT]
                    extra = ws[_WAIT_LIMIT:]
                    inst.sync_info = mybir.SyncInfo(
                        on_wait=keep, on_update=list(si.on_update)
                    )
                    for k, w in enumerate(extra):
                        nop = mybir.InstNoOp(
                            name=f"{inst.name}-w{k}",
                            engine=inst.engine,
                            bass_nofuse=True,
                            sync_info=mybir.SyncInfo(on_wait=[w], on_update=[]),
                        )
                        nc.register_instruction(nop, overwrite=True)
                        insts.insert(i, nop)
                        i += 1
                i += 1


def _build(collective: bool = True) -> bass.Bass:
    """collective=False swaps the AllGather for a local DRAM copy so the
    module is single-core simulatable — timing analysis only."""
    nc = bass.Bass(num_devices=NCORES)

    # ---- DRAM I/O (per-core values supplied via in_maps) ----
    sm16_d = nc.dram_tensor("sm16", [128, _SM16], f16, kind="ExternalInput")
    sm32_d = nc.dram_tensor("sm32", [128, 4], f32, kind="ExternalInput")
    sup_d = nc.dram_tensor("supT", [3, 128, 3, 4, N], f16, kind="ExternalInput")
    wt_d = nc.dram_tensor("wt", [128, KT, JS], f16, kind="ExternalInput")
    out_d = nc.dram_tensor("out", [B, JS], f32, kind="ExternalOutput")

    with PatchedTileContext(nc) as tc:
        from contextlib import ExitStack

        with ExitStack() as ctx:
            const_p = ctx.enter_context(tc.tile_pool(name="const", bufs=1))
            sup_p = ctx.enter_context(tc.tile_pool(name="sup", bufs=1))
            xm_p = ctx.enter_context(tc.tile_pool(name="xm", bufs=2))
            sc_p = ctx.enter_context(tc.tile_pool(name="sc", bufs=2))
            fus_p = ctx.enter_context(tc.tile_pool(name="fus", bufs=1))
            fu_p = ctx.enter_context(tc.tile_pool(name="fu", bufs=5))
            acc_p = ctx.enter_context(tc.tile_pool(name="acc", bufs=4, space="PSUM"))
            tr_p = ctx.enter_context(tc.tile_pool(name="tr", bufs=4, space="PSUM"))
            dram_p = ctx.enter_context(tc.tile_pool(name="dram", bufs=1, space="DRAM"))

            # ---- constants / memsets (gpsimd; off the DMA queue) ----
            id128 = const_p.tile([128, 128], f16, tag="id")
            masks.make_identity(nc, id128[:])
            ones40 = const_p.tile([1, 40], f16, tag="ones")
            nc.vector.memset(ones40[:], 1.0)

            adv1 = const_p.tile([128, 5, N], f16, tag="stk")
            dif1 = const_p.tile([128, 1, N], f16, tag="dstk")
            nc.gpsimd.memset(adv1[:], 0.0)
            nc.gpsimd.memset(dif1[:], 0.0)
            # W_f-GEMM stationary [q, col, kt]; cols 8-31 are never DMA'd
            # -> zero them once, early (garbage would NaN the psum)
            gt_all = const_p.tile([128, 40, KT], f16, tag="gt")
            nc.gpsimd.memset(gt_all[:], 0.0)

            # ---- input DMAs: smalls first, then supports one-by-one so
            # L1 pass A can start as soon as support 0 lands ----
            sm16 = const_p.tile([128, _SM16], f16, tag="sm16")
            nc.sync.dma_start(sm16[:], sm16_d[:])
            sm32 = const_p.tile([128, 4], f32, tag="sm32")
            nc.sync.dma_start(sm32[:], sm32_d[:])
            sup_tiles = []
            for s in range(9):
                supb = sup_p.tile([128, 4, N], f16, tag=f"sup{s}")
                sup_tiles.append(supb)
                nc.sync.dma_start(supb[:], sup_d[s // 3][:, s % 3])

            def sup_ap(s, m):
                return sup_tiles[s][:, m, :]

            # W_f shard split over the 3 DMA-capable queues (SP/Pool/ACT);
            # the ACT chunk is held back so it doesn't stall L1 copies
            wt_all = const_p.tile([128, KT, JS], f16, tag="wt")
            nc.sync.dma_start(wt_all[:, 0:24, :], wt_d[:, 0:24, :])
            nc.gpsimd.dma_start(wt_all[:, 24:48, :], wt_d[:, 24:48, :])
            with tc.tile_wait_until(0.025):
                nc.scalar.dma_start(wt_all[:, 48:64, :], wt_d[:, 48:64, :])

            # packed-small views
            def x0m_ap(m):
                return sm16[:, _OFF_X0M + m * FL : _OFF_X0M + (m + 1) * FL]

            def wa1_ap(t):
                return sm16[:, _OFF_WA1 + t * U : _OFF_WA1 + (t + 1) * U]

            def wa2_ap(t, k=128):
                return sm16[0:k, _OFF_WA2 + t * FL : _OFF_WA2 + (t + 1) * FL]

            def wd2_ap(t, k=128):
                return sm16[0:k, _OFF_WD2 + t * FL : _OFF_WD2 + (t + 1) * FL]

            def bf_ap(lo, hi):
                return sm16[0:1, _OFF_BF + lo : _OFF_BF + hi]

            wd1_ap = sm16[0:80, _OFF_WD1 : _OFF_WD1 + U]
            x0t_ap = sm16[0:FL, _OFF_X0T : _OFF_X0T + N]
            ba1 = sm32[0:U, 0:1]
            bd1 = sm32[0:U, 1:2]
            ba2 = sm32[0:FL, 2:3]
            bd2 = sm32[0:FL, 3:4]

            # x0t into the L1 stacks' mat-0 slots (on-chip copies)
            nc.scalar.copy(adv1[_l1_base(0) : _l1_base(0) + FL, 0, :], x0t_ap)
            nc.vector.tensor_copy(dif1[32 : 32 + FL, 0, :], x0t_ap)

            def slot1(s, which):
                if s < 8:
                    j = 2 * s + which
                    return adv1[_l1_base(j) : _l1_base(j) + FL, j // 4, :]
                # diff mats: x1 -> base 0, x0 -> 32, x2 -> 64
                return dif1[64 * (which - 1) : 64 * (which - 1) + FL, 0, :]

            def cheb(fin, x_m_fn, in1_fn, slot, idb,
                     order_a=tuple(range(9)), order_b=tuple(range(9))):
                """Chebyshev passes for all 9 supports.

                x_m_fn(s, m): [128, fin] stationary input tile for x1.
                in1_fn(s): [fin, N] fp16 transposed x0 (x2 = 2*S@x1 - x0).
                slot(s, which): destination AP for x1/x2 (fp16 stacks).
                idb(s): base partition of slot(s, 1) for the transpose id.
                """
                # pass A: x1 = S @ x0 for every support; PE stays dense
                for s in order_a:
                    ps1 = acc_p.tile([U, N], f32, tag="ps")
                    for m in range(4):
                        nc.tensor.matmul(
                            ps1[0:fin, :], x_m_fn(s, m), sup_ap(s, m),
                            start=(m == 0), stop=(m == 3),
                        )
                    tgt1 = slot(s, 1)
                    if s % 2 == 0:
                        nc.vector.tensor_copy(tgt1, ps1[0:fin, :])
                    else:
                        nc.scalar.copy(tgt1, ps1[0:fin, :])
                # pass B: transpose x1, then x2' = 2*(S@x1) - x0
                for s in order_b:
                    tgt1 = slot(s, 1)
                    bb = idb(s)
                    x1m = xm_p.tile([128, 4, U], f16, tag="x1m")
                    for m in range(4):
                        pt = tr_p.tile([128, U], f16, tag="pt")
                        nc.tensor.transpose(
                            pt[:, 0:fin],
                            tgt1[:, m * 128 : (m + 1) * 128],
                            id128[bb : bb + fin, bb : bb + fin],
                        )
                        if m % 2 == 0:
                            nc.vector.tensor_copy(x1m[:, m, 0:fin], pt[:, 0:fin])
                        else:
                            nc.scalar.copy(x1m[:, m, 0:fin], pt[:, 0:fin])
                    ps2 = acc_p.tile([U, N], f32, tag="ps")
                    for m in range(4):
                        nc.tensor.matmul(
                            ps2[0:fin, :], x1m[:, m, 0:fin], sup_ap(s, m),
                            start=(m == 0), stop=(m == 3),
                        )
                    nc.vector.scalar_tensor_tensor(
                        slot(s, 2), ps2[0:fin, :], 2.0, in1_fn(s),
                        ALU.mult, ALU.subtract,
                    )

            # ---- Layer 1 (fin=16) ----
            cheb(
                FL,
                lambda s, m: x0m_ap(m),
                lambda s: x0t_ap,
                slot1,
                lambda s: 0 if s == 8 else _l1_base(2 * s + 1),
            )

            # L1 GEMMs -> c1 = tanh(xs @ W1 + b1), transposed [U, N]
            pc1a = acc_p.tile([U, N], f32, tag="ps")
            for t in range(4):
                nc.tensor.matmul(
                    pc1a[:], wa1_ap(t), adv1[:, t, :], start=(t == 0), stop=False
                )
            nc.tensor.matmul(
                pc1a[:],
                sm16[64:80, _OFF_WA1 + 4 * U : _OFF_WA1 + 5 * U],
                adv1[64:80, 4, :],
                start=False, stop=True,
            )
            pc1d = acc_p.tile([U, N], f32, tag="ps")
            nc.tensor.matmul(pc1d[:], wd1_ap, dif1[0:80, 0, :], start=True, stop=True)

            adv2 = const_p.tile([128, 9, N], f16, tag="stk")
            dif2 = const_p.tile([128, 2, N], f16, tag="dstk")
            nc.scalar.activation(adv2[0:U, 0, :], pc1a[:], AF.Tanh, bias=ba1)
            nc.scalar.activation(dif2[0:U, 0, :], pc1d[:], AF.Tanh, bias=bd1)

            # transpose c1 -> node-major stationary [128, 4, U]
            c1a_m = xm_p.tile([128, 4, U], f16, tag="c1m")
            c1d_m = xm_p.tile([128, 4, U], f16, tag="c1m")
            for src, dst in ((adv2, c1a_m), (dif2, c1d_m)):
                for m in range(4):
                    pt = tr_p.tile([128, U], f16, tag="pt")
                    nc.tensor.transpose(
                        pt[:], src[0:U, 0, m * 128 : (m + 1) * 128], id128[0:U, 0:U]
                    )
                    if m % 2 == 0:
                        nc.vector.tensor_copy(dst[:, m, :], pt[:])
                    else:
                        nc.scalar.copy(dst[:, m, :], pt[:])

            # ---- Layer 2 (fin=64) ----
            def slot2(s, which):
                if s < 8:
                    j = 2 * s + which
                    return adv2[U * (j % 2) : U * (j % 2) + U, j // 2, :]
                return dif2[U * (which % 2) : U * (which % 2) + U, which // 2, :]

            # diff (s=8) first in pass B so its grad chain overlaps the
            # adv supports' tail
            cheb(
                U,
                lambda s, m: (c1a_m if s < 8 else c1d_m)[:, m, :],
                lambda s: adv2[0:U, 0, :] if s < 8 else dif2[0:U, 0, :],
                slot2,
                lambda s: U,
                order_b=(8, 0, 1, 2, 3, 4, 5, 6, 7),
            )

            # L2 GEMMs -> grads, feature-major [FL, N] fp16. W_d2/b_d2 and
            # W_a2/b_a2 are host-negated so tanh lands the grad sign; diff
            # still needs the 0.1 coefficient (folded into its copies).
            g_st = fus_p.tile([128, 2, 4, FL], f16, tag="gst")
            pgd = acc_p.tile([U, N], f32, tag="ps")
            nc.tensor.matmul(
                pgd[0:FL, :], wd2_ap(0), dif2[:, 0, :], start=True, stop=False
            )
            nc.tensor.matmul(
                pgd[0:FL, :], wd2_ap(1, U), dif2[0:U, 1, :], start=False, stop=True
            )
            gd_t = sc_p.tile([FL, N], f16, tag="x1tsc")
            nc.scalar.activation(gd_t[:], pgd[0:FL, :], AF.Tanh, bias=bd2)
            for m in range(4):
                pt = tr_p.tile([128, U], f16, tag="pt")
                nc.tensor.transpose(
                    pt[:, 0:FL], gd_t[:, m * 128 : (m + 1) * 128], id128[0:FL, 0:FL]
                )
                nc.vector.tensor_scalar_mul(g_st[:, 0, m, :], pt[:, 0:FL], COEFF)

            pga = acc_p.tile([U, N], f32, tag="ps")
            for t in range(9):
                kk = 128 if t < 8 else U
                nc.tensor.matmul(
                    pga[0:FL, :], wa2_ap(t, kk), adv2[0:kk, t, :],
                    start=(t == 0), stop=(t == 8),
                )
            ga_t = sc_p.tile([FL, N], f16, tag="x1tsc")
            nc.scalar.activation(ga_t[:], pga[0:FL, :], AF.Tanh, bias=ba2)
            for m in range(4):
                pt = tr_p.tile([128, U], f16, tag="pt")
                nc.tensor.transpose(
                    pt[:, 0:FL], ga_t[:, m * 128 : (m + 1) * 128], id128[0:FL, 0:FL]
                )
                if m % 2 == 0:
                    nc.vector.tensor_copy(g_st[:, 1, m, :], pt[:, 0:FL])
                else:
                    nc.scalar.copy(g_st[:, 1, m, :], pt[:, 0:FL])

            # ---- AllGather of node-major grads: agin[r, p, m, f] ----
            agin = dram_p.tile([2, 128, 4, FL], f16)
            agout = dram_p.tile([NCORES, 2, 128, 4, FL], f16)
            nc.sync.dma_start(agin[0].rearrange("p m f -> p (m f)"),
                              g_st[:, 0].rearrange("p m f -> p (m f)"))
            nc.scalar.dma_start(agin[1].rearrange("p m f -> p (m f)"),
                                g_st[:, 1].rearrange("p m f -> p (m f)"))
            if collective:
                nc.gpsimd.collective_compute(
                    "AllGather",
                    ALU.bypass,
                    replica_groups=[list(range(NCORES))],
                    ins=[agin.opt()],
                    outs=[agout.opt()],
                )
            else:
                for r in range(NCORES):
                    nc.gpsimd.dma_start(agout[r], agin[:])

            # ---- W_f phase ----
            # Gathered grads land directly in the stationary layout:
            # gt_all[q, col, kt] with kt = m*16+f <-> k = (m*128+q)*16+f;
            # the wt host layout uses the same enumeration. Diff grads ->
            # cols 0-7, adv -> cols 32-39. Both DMAs are 3-dim APs with a
            # contiguous 64-element last dim.
            nc.sync.dma_start(
                gt_all[:, 0:8, :],
                agout[:, 0].rearrange("c p m f -> p c (m f)"),
            )
            nc.scalar.dma_start(
                gt_all[:, 32:40, :],
                agout[:, 1].rearrange("c p m f -> p c (m f)"),
            )

            # Half 1's GEMM completes first so its fusion chain runs under
            # half 2's GEMM; each half is 64 matmuls + a bias row.
            def fusion(ps, h):
                # only one PSUM operand allowed per DVE op -> stage X_adv
                xa = fu_p.tile([B, 512], f16, tag="fu")
                nc.scalar.copy(xa[:], ps[32 : 32 + B, :])
                ssum = fu_p.tile([B, 512], f16, tag="fu")
                nc.vector.tensor_add(ssum[:], ps[0:B, :], xa[:])
                d = fu_p.tile([B, 512], f16, tag="fu")
                nc.vector.tensor_sub(d[:], ps[0:B, :], xa[:])
                z = fu_p.tile([B, 512], f16, tag="fu")
                nc.scalar.activation(z[:], ssum[:], AF.Sigmoid)
                zd = fu_p.tile([B, 512], f16, tag="fu")
                nc.vector.tensor_mul(zd[:], z[:], d[:])
                o = fus_p.tile([B, 512], f32, tag="fo")
                nc.vector.tensor_add(o[:], zd[:], ps[32 : 32 + B, :])
                nc.sync.dma_start(out_d[:, h * 512 : (h + 1) * 512], o[:])

            psX1 = acc_p.tile([40, 512], f32, tag="ps")
            psX2 = acc_p.tile([40, 512], f32, tag="ps")
            for kt in range(KT):
                nc.tensor.matmul(
                    psX1[:], gt_all[:, :, kt], wt_all[:, kt, 0:512],
                    start=(kt == 0), stop=False, skip_group_check=True,
                )
            nc.tensor.matmul(
                psX1[:], ones40[:], bf_ap(0, 512),
                start=False, stop=True, skip_group_check=True,
            )
            fusion(psX1, 0)
            for kt in range(KT):
                nc.tensor.matmul(
                    psX2[:], gt_all[:, :, kt], wt_all[:, kt, 512:JS],
                    start=(kt == 0), stop=False, skip_group_check=True,
                )
            nc.tensor.matmul(
                psX2[:], ones40[:], bf_ap(512, JS),
                start=False, stop=True, skip_group_check=True,
            )
            fusion(psX2, 1)

    _split_excess_waits(nc)
    return nc


def _prep_in_maps(inputs: dict) -> list[dict]:
    y = np.asarray(inputs["y"], np.float32)
    sd = np.asarray(inputs["supports_diff"], np.float32)
    sa = np.asarray(inputs["supports_adv"], np.float32)
    W_d1 = np.asarray(inputs["W_d1"], np.float32)
    W_d2 = -np.asarray(inputs["W_d2"], np.float32)
    W_a1 = np.asarray(inputs["W_a1"], np.float32)
    W_a2 = -np.asarray(inputs["W_a2"], np.float32)
    W_f = np.asarray(inputs["W_f"], np.float32)
    b_f = np.asarray(inputs["b_f"], np.float32)

    # supports, transposed, node-tile-major, 3 per DMA block:
    # supT[b, p, si, m, n] = S_{3b+si}.T[m*128+p, n]
    supT = np.empty((3, 128, 3, 4, N), np.float16)
    for s in range(9):
        Ssrc = sa[s] if s < 8 else sd[0]
        st = Ssrc.T.astype(np.float16)  # [m, n]
        supT[s // 3, :, s % 3] = st.reshape(4, 128, N).transpose(1, 0, 2)

    def perm_pad(W, fin, M, fout, ntiles):
        # reference row (f, m) -> packed row m*fin+f, zero-padded to tiles
        Wp = W.reshape(fin, M, fout).transpose(1, 0, 2).reshape(fin * M, fout)
        pad = np.zeros((ntiles * 128, fout), np.float16)
        pad[: fin * M] = Wp.astype(np.float16)
        return pad.reshape(ntiles, 128, fout)

    wa2 = perm_pad(W_a2, U, 17, FL, 9)
    wd2 = perm_pad(W_d2, U, 3, FL, 2)

    # L1 adv weights: mat j at tile j//4, base _l1_base(j)
    wa1 = np.zeros((5, 128, U), np.float16)
    for j in range(17):
        base = _l1_base(j)
        wa1[j // 4, base : base + FL, :] = W_a1[np.arange(FL) * 17 + j, :].astype(
            np.float16
        )
    # L1 diff weights: x1(m=1)@0, x0(m=0)@32, x2(m=2)@64
    wd1 = np.zeros((80, U), np.float16)
    for j, base in ((1, 0), (0, 32), (2, 64)):
        wd1[base : base + FL, :] = W_d1[np.arange(FL) * 3 + j, :].astype(np.float16)

    sm32 = np.zeros((128, 4), np.float32)
    sm32[0:U, 0] = np.asarray(inputs["b_a1"], np.float32)
    sm32[0:U, 1] = np.asarray(inputs["b_d1"], np.float32)
    sm32[0:FL, 2] = -np.asarray(inputs["b_a2"], np.float32)
    sm32[0:FL, 3] = -np.asarray(inputs["b_d2"], np.float32)

    # wt[q, m*16+f, j] = W_f.T[(m*128+q)*FL + f, c*JS+j]  (kt = m*16+f)
    WT = W_f.T.astype(np.float16)  # [k_orig = n*FL+f, j_global]
    in_maps = []
    for c in range(NCORES):
        x0 = y[c].reshape(N, FL)  # [node, f]
        x0m = x0.reshape(4, 128, FL).transpose(1, 0, 2).astype(np.float16)
        x0t = x0.T.astype(np.float16)

        sm16 = np.zeros((128, _SM16), np.float16)
        sm16[:, _OFF_X0M : _OFF_X0M + 64] = x0m.reshape(128, 64)
        sm16[:, _OFF_WA1 : _OFF_WA1 + 5 * U] = wa1.transpose(1, 0, 2).reshape(
            128, 5 * U
        )
        sm16[0:80, _OFF_WD1 : _OFF_WD1 + U] = wd1
        sm16[:, _OFF_WA2 : _OFF_WA2 + 9 * FL] = wa2.transpose(1, 0, 2).reshape(
            128, 9 * FL
        )
        sm16[:, _OFF_WD2 : _OFF_WD2 + 2 * FL] = wd2.transpose(1, 0, 2).reshape(
            128, 2 * FL
        )
        sm16[0, _OFF_BF : _OFF_BF + JS] = b_f[c * JS : (c + 1) * JS].astype(
            np.float16
        )
        sm16[0:FL, _OFF_X0T : _OFF_X0T + N] = x0t

        # [(m q f), j] -> [q, m, f, j] -> [q, m*16+f, j]
        wt = np.ascontiguousarray(
            WT[:, c * JS : (c + 1) * JS]
            .reshape(4, 128, FL, JS)
            .transpose(1, 0, 2, 3)
            .reshape(128, KT, JS)
        )
        in_maps.append({"sm16": sm16, "sm32": sm32, "supT": supT, "wt": wt})
    return in_maps


_CACHE: dict = {}


def _get_nc() -> bass.Bass:
    if "nc" not in _CACHE:
        _CACHE["nc"] = _build()
    return _CACHE["nc"]


def run(inputs: dict, trace: bool = False):
    """Run on the 8 cores; returns (full_output, BassKernelResults)."""
    in_maps = _prep_in_maps(inputs)
    nc = _get_nc()
    kw = {}
    if trace:
        kw = dict(trace=True, trace_cores=list(range(NCORES)), stitch_traces=False)
    res = run_bass_kernel_spmd(nc, in_maps, core_ids=list(range(NCORES)), **kw)
    out = np.concatenate(
        [res.results[c]["out"] for c in range(NCORES)], axis=1
    ).astype(np.float32)
    return out, res


def kernel(**inputs) -> np.ndarray:
    out, _ = run(inputs)
    return out


# revision 16
# speedup vs baseline: 1.1089x; 1.1089x over previous
"""Trainium2 Bass kernel for nn_ODEFunc (gnn_message_passing, 8 cores).

Strategy:
  - Batch-parallel branches: core b computes batch b's diff+adv gconv
    branches (all 9 support matrices stream through the PE as fp16).
  - Transposed-mat layout [feature, node]; PE transposes flip layouts for
    the Chebyshev recurrence. Two-pass structure per layer (all x1
    matmuls, then per-support transpose+x2) keeps the PE dense.
  - L2 weights/biases are negated on the host so tanh emits the final
    grad sign for free; the diff branch's extra 0.1 factor is one DVE op.
  - Grads stay feature-major [16, 512] fp16 -> AllGather -> agout.
  - W_f GEMM contraction is enumerated k = f*512 + n so the gathered
    grads DMA straight into the stationary layout gt_all[128, KT, 40]
    (two strided DMAs, no transposes, no staging); wt is host-permuted
    to the same k order. psum[40, 1024]: X_diff rows 0-7, X_adv 32-39;
    cols 8-31 are memset-zero garbage lanes.
  - W_f shard (fp16, 16 MB) is split into 4 DMAs on 4 engine queues so
    no single queue serializes behind it.
  - Gated fusion (sigmoid) on-chip; core c returns out[:, c*1024:...].

Mat slot bases: matmul operands must start at partition 0/32/64 (other
engines also allow 96). L1 packs 4 16-row mats per 128-row K-tile; x1
mats (which feed PE transposes) go to bases 0/32, x0/x2 to 64/96, with
the L1 weight rows permuted on the host to match. L2 mats are 64-row:
x1 at base 64, x0/x2 at 0, all legal.
"""

import sys

sys.path.insert(0, "/opt/trn_rl_repo")

import numpy as np

import concourse.bass as bass
import concourse.mybir as mybir
from concourse import masks
from concourse.bass_utils import run_bass_kernel_spmd
from concourse.tile import TileContext
from concourse.vector_clock import ScopedClock

N = 512          # nodes
FL = 16          # latent
U = 64           # units
B = 8            # batch
HID = N * FL     # 8192
COEFF = 0.1
NCORES = 8
JS = HID // NCORES  # 1024 output columns per core
KT = HID // 128     # 64 contraction tiles for the W_f GEMM

f16 = mybir.dt.float16
f32 = mybir.dt.float32
AF = mybir.ActivationFunctionType
ALU = mybir.AluOpType


# L1 within-tile base for mat j (16-row mats in 128-row tiles of 4):
# x1 mats (odd j) at 0/32 so they are legal PE-transpose inputs.
def _l1_base(j):
    return {1: 0, 3: 32, 0: 64, 2: 96}[j % 4]


# smalls_f16 packed free-dim offsets (elements)
_OFF_X0M = 0          # [128, 4*16]
_OFF_WA1 = 64         # [128, 5*64]
_OFF_WD1 = 384        # [80(->128), 64]
_OFF_WA2 = 448        # [128, 9*16]
_OFF_WD2 = 592        # [128, 2*16]
_OFF_BF = 624         # [1, 1024]
_OFF_X0T = 1648       # [16, 512]
_SM16 = 2160


class PatchedTileContext(TileContext):
    """Tail drain with at most one sem wait per instruction.

    The walrus build here rejects Drain instructions carrying >2 sync
    waits ("Too many sync wait commands"). Spread the global-clock waits
    over individual SP nops ahead of the drain.
    """

    def _drain_and_barrier(self, tick_clock, wait_clock):
        nc = self.nc
        probe = nc.sync.nop(nofuse=True)
        wait_clock.add_sem_waits(
            probe.ins, ScopedClock({None: tick_clock.global_clock})
        )
        si = probe.ins.sync_info
        ws = list(si.on_wait) if si is not None else []
        if len(ws) > 1:
            probe.ins.sync_info = mybir.SyncInfo(
                on_wait=ws[:1], on_update=list(si.on_update)
            )
            for w in ws[1:]:
                n2 = nc.sync.nop(nofuse=True)
                n2.ins.sync_info = mybir.SyncInfo(on_wait=[w], on_update=[])
        nc.sync.drain()
        nc.all_engine_barrier()
        popped = nc._tile_sem_poison_stack.pop()
        assert popped is self._sem_poison
        nc.clear_and_free_semaphores(list(self.sems.allocated().values()))
        nc.all_engine_barrier()


_WAIT_LIMIT = 1


def _split_excess_waits(nc: bass.Bass) -> None:
    """Move sync waits beyond _WAIT_LIMIT onto same-engine NOPs inserted
    just before the carrying instruction (this walrus build has tiny
    setupSyncWait budgets for DMA/collective/drain instruction formats)."""
    for fn in nc.m.functions:
        for bb in fn.blocks:
            insts = bb.instructions
            i = 0
            while i < len(insts):
                inst = insts[i]
                si = inst.sync_info
                ws = list(si.on_wait) if si is not None and si.on_wait else []
                if len(ws) > _WAIT_LIMIT and type(inst).__name__ != "InstNoOp":
                    keep = ws[:_WAIT_LIMIT]
                    extra = ws[_WAIT_LIMIT:]
                    inst.sync_info = mybir.SyncInfo(
                        on_wait=keep, on_update=list(si.on_update)
                    )
                    for k, w in enumerate(extra):
                        nop = mybir.InstNoOp(
                            name=f"{inst.name}-w{k}",
                            engine=inst.engine,
                            bass_nofuse=True,
                            sync_info=mybir.SyncInfo(on_wait=[w], on_update=[]),
                        )
                        nc.register_instruction(nop, overwrite=True)
                        insts.insert(i, nop)
                        i += 1
                i += 1


def _build(collective: bool = True) -> bass.Bass:
    """collective=False swaps the AllGather for a local DRAM copy so the
    module is single-core simulatable — timing analysis only."""
    nc = bass.Bass(num_devices=NCORES)

    # ---- DRAM I/O (per-core values supplied via in_maps) ----
    sm16_d = nc.dram_tensor("sm16", [128, _SM16], f16, kind="ExternalInput")
    sm32_d = nc.dram_tensor("sm32", [128, 4], f32, kind="ExternalInput")
    sup_d = nc.dram_tensor("supT", [3, 128, 3, 4, N], f16, kind="ExternalInput")
    wt_d = nc.dram_tensor("wt", [128, KT, JS], f16, kind="ExternalInput")
    out_d = nc.dram_tensor("out", [B, JS], f32, kind="ExternalOutput")

    with PatchedTileContext(nc) as tc:
        from contextlib import ExitStack

        with ExitStack() as ctx:
            const_p = ctx.enter_context(tc.tile_pool(name="const", bufs=1))
            sup_p = ctx.enter_context(tc.tile_pool(name="sup", bufs=1))
            xm_p = ctx.enter_context(tc.tile_pool(name="xm", bufs=2))
            sc_p = ctx.enter_context(tc.tile_pool(name="sc", bufs=2))
            fus_p = ctx.enter_context(tc.tile_pool(name="fus", bufs=1))
            fu_p = ctx.enter_context(tc.tile_pool(name="fu", bufs=5))
            acc_p = ctx.enter_context(tc.tile_pool(name="acc", bufs=4, space="PSUM"))
            tr_p = ctx.enter_context(tc.tile_pool(name="tr", bufs=4, space="PSUM"))
            dram_p = ctx.enter_context(tc.tile_pool(name="dram", bufs=1, space="DRAM"))

            # ---- constants / memsets (gpsimd; off the DMA queue) ----
            id128 = const_p.tile([128, 128], f16, tag="id")
            masks.make_identity(nc, id128[:])
            ones40 = const_p.tile([1, 40], f16, tag="ones")
            nc.vector.memset(ones40[:], 1.0)

            adv1 = const_p.tile([128, 5, N], f16, tag="stk")
            dif1 = const_p.tile([128, 1, N], f16, tag="dstk")
            nc.gpsimd.memset(adv1[:], 0.0)
            nc.gpsimd.memset(dif1[:], 0.0)
            # W_f-GEMM stationary [q, col, kt]; cols 8-31 are never DMA'd
            # -> zero them once, early (garbage would NaN the psum)
            gt_all = const_p.tile([128, 40, KT], f16, tag="gt")
            nc.gpsimd.memset(gt_all[:], 0.0)

            # ---- input DMAs: smalls first, then supports one-by-one so
            # L1 pass A can start as soon as support 0 lands ----
            sm16 = const_p.tile([128, _SM16], f16, tag="sm16")
            nc.sync.dma_start(sm16[:], sm16_d[:])
            sm32 = const_p.tile([128, 4], f32, tag="sm32")
            nc.sync.dma_start(sm32[:], sm32_d[:])
            sup_tiles = []
            for s in range(9):
                supb = sup_p.tile([128, 4, N], f16, tag=f"sup{s}")
                sup_tiles.append(supb)
                nc.sync.dma_start(supb[:], sup_d[s // 3][:, s % 3])

            def sup_ap(s, m):
                return sup_tiles[s][:, m, :]

            # W_f shard split over the 3 DMA-capable queues (SP/Pool/ACT);
            # the ACT chunk is held back so it doesn't stall L1 copies
            wt_all = const_p.tile([128, KT, JS], f16, tag="wt")
            nc.sync.dma_start(wt_all[:, 0:24, :], wt_d[:, 0:24, :])
            nc.gpsimd.dma_start(wt_all[:, 24:52, :], wt_d[:, 24:52, :])
            with tc.tile_wait_until(0.058):
                nc.scalar.dma_start(wt_all[:, 52:64, :], wt_d[:, 52:64, :])

            # packed-small views
            def x0m_ap(m):
                return sm16[:, _OFF_X0M + m * FL : _OFF_X0M + (m + 1) * FL]

            def wa1_ap(t):
                return sm16[:, _OFF_WA1 + t * U : _OFF_WA1 + (t + 1) * U]

            def wa2_ap(t, k=128):
                return sm16[0:k, _OFF_WA2 + t * FL : _OFF_WA2 + (t + 1) * FL]

            def wd2_ap(t, k=128):
                return sm16[0:k, _OFF_WD2 + t * FL : _OFF_WD2 + (t + 1) * FL]

            def bf_ap(lo, hi):
                return sm16[0:1, _OFF_BF + lo : _OFF_BF + hi]

            wd1_ap = sm16[0:80, _OFF_WD1 : _OFF_WD1 + U]
            x0t_ap = sm16[0:FL, _OFF_X0T : _OFF_X0T + N]
            ba1 = sm32[0:U, 0:1]
            bd1 = sm32[0:U, 1:2]
            ba2 = sm32[0:FL, 2:3]
            bd2 = sm32[0:FL, 3:4]

            # x0t into the L1 stacks' mat-0 slots (on-chip copies)
            nc.scalar.copy(adv1[_l1_base(0) : _l1_base(0) + FL, 0, :], x0t_ap)
            nc.vector.tensor_copy(dif1[32 : 32 + FL, 0, :], x0t_ap)

            def slot1(s, which):
                if s < 8:
                    j = 2 * s + which
                    return adv1[_l1_base(j) : _l1_base(j) + FL, j // 4, :]
                # diff mats: x1 -> base 0, x0 -> 32, x2 -> 64
                return dif1[64 * (which - 1) : 64 * (which - 1) + FL, 0, :]

            def cheb(fin, x_m_fn, in1_fn, slot, idb,
                     order_a=tuple(range(9)), order_b=tuple(range(9))):
                """Chebyshev passes for all 9 supports.

                x_m_fn(s, m): [128, fin] stationary input tile for x1.
                in1_fn(s): [fin, N] fp16 transposed x0 (x2 = 2*S@x1 - x0).
                slot(s, which): destination AP for x1/x2 (fp16 stacks).
                idb(s): base partition of slot(s, 1) for the transpose id.
                """
                # pass A: x1 = S @ x0 for every support; PE stays dense
                for s in order_a:
                    ps1 = acc_p.tile([U, N], f32, tag="ps")
                    for m in range(4):
                        nc.tensor.matmul(
                            ps1[0:fin, :], x_m_fn(s, m), sup_ap(s, m),
                            start=(m == 0), stop=(m == 3),
                        )
                    tgt1 = slot(s, 1)
                    if s % 2 == 0:
                        nc.vector.tensor_copy(tgt1, ps1[0:fin, :])
                    else:
                        nc.scalar.copy(tgt1, ps1[0:fin, :])
                # pass B: transpose x1, then x2' = 2*(S@x1) - x0
                for s in order_b:
                    tgt1 = slot(s, 1)
                    bb = idb(s)
                    x1m = xm_p.tile([128, 4, U], f16, tag="x1m")
                    for m in range(4):
                        pt = tr_p.tile([128, U], f16, tag="pt")
                        nc.tensor.transpose(
                            pt[:, 0:fin],
                            tgt1[:, m * 128 : (m + 1) * 128],
                            id128[bb : bb + fin, bb : bb + fin],
                        )
                        if m % 2 == 0:
                            nc.vector.tensor_copy(x1m[:, m, 0:fin], pt[:, 0:fin])
                        else:
                            nc.scalar.copy(x1m[:, m, 0:fin], pt[:, 0:fin])
                    ps2 = acc_p.tile([U, N], f32, tag="ps")
                    for m in range(4):
                        nc.tensor.matmul(
                            ps2[0:fin, :], x1m[:, m, 0:fin], sup_ap(s, m),
                            start=(m == 0), stop=(m == 3),
                        )
                    nc.vector.scalar_tensor_tensor(
                        slot(s, 2), ps2[0:fin, :], 2.0, in1_fn(s),
                        ALU.mult, ALU.subtract,
                    )

            # ---- Layer 1 (fin=16) ----
            cheb(
                FL,
                lambda s, m: x0m_ap(m),
                lambda s: x0t_ap,
                slot1,
                lambda s: 0 if s == 8 else _l1_base(2 * s + 1),
            )

            # L1 GEMMs -> c1 = tanh(xs @ W1 + b1), transposed [U, N]
            pc1a = acc_p.tile([U, N], f32, tag="ps")
            for t in range(4):
                nc.tensor.matmul(
                    pc1a[:], wa1_ap(t), adv1[:, t, :], start=(t == 0), stop=False
                )
            nc.tensor.matmul(
                pc1a[:],
                sm16[64:80, _OFF_WA1 + 4 * U : _OFF_WA1 + 5 * U],
                adv1[64:80, 4, :],
                start=False, stop=True,
            )
            pc1d = acc_p.tile([U, N], f32, tag="ps")
            nc.tensor.matmul(pc1d[:], wd1_ap, dif1[0:80, 0, :], start=True, stop=True)

            adv2 = const_p.tile([128, 9, N], f16, tag="stk")
            dif2 = const_p.tile([128, 2, N], f16, tag="dstk")
            nc.scalar.activation(adv2[0:U, 0, :], pc1a[:], AF.Tanh, bias=ba1)
            nc.scalar.activation(dif2[0:U, 0, :], pc1d[:], AF.Tanh, bias=bd1)

            # transpose c1 -> node-major stationary [128, 4, U]
            c1a_m = xm_p.tile([128, 4, U], f16, tag="c1m")
            c1d_m = xm_p.tile([128, 4, U], f16, tag="c1m")
            for src, dst in ((adv2, c1a_m), (dif2, c1d_m)):
                for m in range(4):
                    pt = tr_p.tile([128, U], f16, tag="pt")
                    nc.tensor.transpose(
                        pt[:], src[0:U, 0, m * 128 : (m + 1) * 128], id128[0:U, 0:U]
                    )
                    if m % 2 == 0:
                        nc.vector.tensor_copy(dst[:, m, :], pt[:])
                    else:
                        nc.scalar.copy(dst[:, m, :], pt[:])

            # ---- Layer 2 (fin=64) ----
            def slot2(s, which):
                if s < 8:
                    j = 2 * s + which
                    return adv2[U * (j % 2) : U * (j % 2) + U, j // 2, :]
                return dif2[U * (which % 2) : U * (which % 2) + U, which // 2, :]

            # diff (s=8) first in pass B so its grad chain overlaps the
            # adv supports' tail
            cheb(
                U,
                lambda s, m: (c1a_m if s < 8 else c1d_m)[:, m, :],
                lambda s: adv2[0:U, 0, :] if s < 8 else dif2[0:U, 0, :],
                slot2,
                lambda s: U,
                order_b=(8, 0, 1, 2, 3, 4, 5, 6, 7),
            )

            # L2 GEMMs -> grads, feature-major [FL, N] fp16. W_d2/b_d2 and
            # W_a2/b_a2 are host-negated so tanh lands the grad sign; diff
            # still needs the 0.1 coefficient (folded into its copies).
            g_st = fus_p.tile([128, 2, 4, FL], f16, tag="gst")
            pgd = acc_p.tile([U, N], f32, tag="ps")
            nc.tensor.matmul(
                pgd[0:FL, :], wd2_ap(0), dif2[:, 0, :], start=True, stop=False
            )
            nc.tensor.matmul(
                pgd[0:FL, :], wd2_ap(1, U), dif2[0:U, 1, :], start=False, stop=True
            )
            gd_t = sc_p.tile([FL, N], f16, tag="x1tsc")
            nc.scalar.activation(gd_t[:], pgd[0:FL, :], AF.Tanh, bias=bd2)
            for m in range(4):
                pt = tr_p.tile([128, U], f16, tag="pt")
                nc.tensor.transpose(
                    pt[:, 0:FL], gd_t[:, m * 128 : (m + 1) * 128], id128[0:FL, 0:FL]
                )
                nc.vector.tensor_scalar_mul(g_st[:, 0, m, :], pt[:, 0:FL], COEFF)

            pga = acc_p.tile([U, N], f32, tag="ps")
            for t in range(9):
                kk = 128 if t < 8 else U
                nc.tensor.matmul(
                    pga[0:FL, :], wa2_ap(t, kk), adv2[0:kk, t, :],
                    start=(t == 0), stop=(t == 8),
                )
            ga_t = sc_p.tile([FL, N], f16, tag="x1tsc")
            nc.scalar.activation(ga_t[:], pga[0:FL, :], AF.Tanh, bias=ba2)
            for m in range(4):
                pt = tr_p.tile([128, U], f16, tag="pt")
                nc.tensor.transpose(
                    pt[:, 0:FL], ga_t[:, m * 128 : (m + 1) * 128], id128[0:FL, 0:FL]
                )
                if m % 2 == 0:
                    nc.vector.tensor_copy(g_st[:, 1, m, :], pt[:, 0:FL])
                else:
                    nc.scalar.copy(g_st[:, 1, m, :], pt[:, 0:FL])

            # ---- AllGather of node-major grads: agin[r, p, m, f] ----
            agin = dram_p.tile([2, 128, 4, FL], f16)
            agout = dram_p.tile([NCORES, 2, 128, 4, FL], f16)
            nc.sync.dma_start(agin[0].rearrange("p m f -> p (m f)"),
                              g_st[:, 0].rearrange("p m f -> p (m f)"))
            nc.scalar.dma_start(agin[1].rearrange("p m f -> p (m f)"),
                                g_st[:, 1].rearrange("p m f -> p (m f)"))
            if collective:
                nc.gpsimd.collective_compute(
                    "AllGather",
                    ALU.bypass,
                    replica_groups=[list(range(NCORES))],
                    ins=[agin.opt()],
                    outs=[agout.opt()],
                )
            else:
                for r in range(NCORES):
                    nc.gpsimd.dma_start(agout[r], agin[:])

            # ---- W_f phase ----
            # Gathered grads land directly in the stationary layout:
            # gt_all[q, col, kt] with kt = m*16+f <-> k = (m*128+q)*16+f;
            # the wt host layout uses the same enumeration. Diff grads ->
            # cols 0-7, adv -> cols 32-39. Both DMAs are 3-dim APs with a
            # contiguous 64-element last dim.
            nc.sync.dma_start(
                gt_all[:, 0:8, :],
                agout[:, 0].rearrange("c p m f -> p c (m f)"),
            )
            nc.scalar.dma_start(
                gt_all[:, 32:40, :],
                agout[:, 1].rearrange("c p m f -> p c (m f)"),
            )

            # Half 1's GEMM completes first so its fusion chain runs under
            # half 2's GEMM; each half is 64 matmuls + a bias row.
            def fusion(ps, h):
                # only one PSUM operand allowed per DVE op -> stage X_adv
                xa = fu_p.tile([B, 512], f16, tag="fu")
                nc.scalar.copy(xa[:], ps[32 : 32 + B, :])
                ssum = fu_p.tile([B, 512], f16, tag="fu")
                nc.vector.tensor_add(ssum[:], ps[0:B, :], xa[:])
                d = fu_p.tile([B, 512], f16, tag="fu")
                nc.vector.tensor_sub(d[:], ps[0:B, :], xa[:])
                z = fu_p.tile([B, 512], f16, tag="fu")
                nc.scalar.activation(z[:], ssum[:], AF.Sigmoid)
                zd = fu_p.tile([B, 512], f16, tag="fu")
                nc.vector.tensor_mul(zd[:], z[:], d[:])
                o = fus_p.tile([B, 512], f32, tag="fo")
                nc.vector.tensor_add(o[:], zd[:], ps[32 : 32 + B, :])
                nc.sync.dma_start(out_d[:, h * 512 : (h + 1) * 512], o[:])

            psX1 = acc_p.tile([40, 512], f32, tag="ps")
            psX2 = acc_p.tile([40, 512], f32, tag="ps")
            for kt in range(KT):
                nc.tensor.matmul(
                    psX1[:], gt_all[:, :, kt], wt_all[:, kt, 0:512],
                    start=(kt == 0), stop=False, skip_group_check=True,
                )
            nc.tensor.matmul(
                psX1[:], ones40[:], bf_ap(0, 512),
                start=False, stop=True, skip_group_check=True,
            )
            fusion(psX1, 0)
            for kt in range(KT):
                nc.tensor.matmul(
                    psX2[:], gt_all[:, :, kt], wt_all[:, kt, 512:JS],
                    start=(kt == 0), stop=False, skip_group_check=True,
                )
            nc.tensor.matmul(
                psX2[:], ones40[:], bf_ap(512, JS),
                start=False, stop=True, skip_group_check=True,
            )
            fusion(psX2, 1)

    _split_excess_waits(nc)
    return nc


def _prep_in_maps(inputs: dict) -> list[dict]:
    y = np.asarray(inputs["y"], np.float32)
    sd = np.asarray(inputs["supports_diff"], np.float32)
    sa = np.asarray(inputs["supports_adv"], np.float32)
    W_d1 = np.asarray(inputs["W_d1"], np.float32)
    W_d2 = -np.asarray(inputs["W_d2"], np.float32)
    W_a1 = np.asarray(inputs["W_a1"], np.float32)
    W_a2 = -np.asarray(inputs["W_a2"], np.float32)
    W_f = np.asarray(inputs["W_f"], np.float32)
    b_f = np.asarray(inputs["b_f"], np.float32)

    # supports, transposed, node-tile-major, 3 per DMA block:
    # supT[b, p, si, m, n] = S_{3b+si}.T[m*128+p, n]
    supT = np.empty((3, 128, 3, 4, N), np.float16)
    for s in range(9):
        Ssrc = sa[s] if s < 8 else sd[0]
        st = Ssrc.T.astype(np.float16)  # [m, n]
        supT[s // 3, :, s % 3] = st.reshape(4, 128, N).transpose(1, 0, 2)

    def perm_pad(W, fin, M, fout, ntiles):
        # reference row (f, m) -> packed row m*fin+f, zero-padded to tiles
        Wp = W.reshape(fin, M, fout).transpose(1, 0, 2).reshape(fin * M, fout)
        pad = np.zeros((ntiles * 128, fout), np.float16)
        pad[: fin * M] = Wp.astype(np.float16)
        return pad.reshape(ntiles, 128, fout)

    wa2 = perm_pad(W_a2, U, 17, FL, 9)
    wd2 = perm_pad(W_d2, U, 3, FL, 2)

    # L1 adv weights: mat j at tile j//4, base _l1_base(j)
    wa1 = np.zeros((5, 128, U), np.float16)
    for j in range(17):
        base = _l1_base(j)
        wa1[j // 4, base : base + FL, :] = W_a1[np.arange(FL) * 17 + j, :].astype(
            np.float16
        )
    # L1 diff weights: x1(m=1)@0, x0(m=0)@32, x2(m=2)@64
    wd1 = np.zeros((80, U), np.float16)
    for j, base in ((1, 0), (0, 32), (2, 64)):
        wd1[base : base + FL, :] = W_d1[np.arange(FL) * 3 + j, :].astype(np.float16)

    sm32 = np.zeros((128, 4), np.float32)
    sm32[0:U, 0] = np.asarray(inputs["b_a1"], np.float32)
    sm32[0:U, 1] = np.asarray(inputs["b_d1"], np.float32)
    sm32[0:FL, 2] = -np.asarray(inputs["b_a2"], np.float32)
    sm32[0:FL, 3] = -np.asarray(inputs["b_d2"], np.float32)

    # wt[q, m*16+f, j] = W_f.T[(m*128+q)*FL + f, c*JS+j]  (kt = m*16+f)
    WT = W_f.T.astype(np.float16)  # [k_orig = n*FL+f, j_global]
    in_maps = []
    for c in range(NCORES):
        x0 = y[c].reshape(N, FL)  # [node, f]
        x0m = x0.reshape(4, 128, FL).transpose(1, 0, 2).astype(np.float16)
        x0t = x0.T.astype(np.float16)

        sm16 = np.zeros((128, _SM16), np.float16)
        sm16[:, _OFF_X0M : _OFF_X0M + 64] = x0m.reshape(128, 64)
        sm16[:, _OFF_WA1 : _OFF_WA1 + 5 * U] = wa1.transpose(1, 0, 2).reshape(
            128, 5 * U
        )
        sm16[0:80, _OFF_WD1 : _OFF_WD1 + U] = wd1
        sm16[:, _OFF_WA2 : _OFF_WA2 + 9 * FL] = wa2.transpose(1, 0, 2).reshape(
            128, 9 * FL
        )
        sm16[:, _OFF_WD2 : _OFF_WD2 + 2 * FL] = wd2.transpose(1, 0, 2).reshape(
            128, 2 * FL
        )
        sm16[0, _OFF_BF : _OFF_BF + JS] = b_f[c * JS : (c + 1) * JS].astype(
            np.float16
        )
        sm16[0:FL, _OFF_X0T : _OFF_X0T + N] = x0t

        # [(m q f), j] -> [q, m, f, j] -> [q, m*16+f, j]
        wt = np.ascontiguousarray(
            WT[:, c * JS : (c + 1) * JS]
            .reshape(4, 128, FL, JS)
            .transpose(1, 0, 2, 3)
            .reshape(128, KT, JS)
        )
        in_maps.append({"sm16": sm16, "sm32": sm32, "supT": supT, "wt": wt})
    return in_maps


_CACHE: dict = {}


def _get_nc() -> bass.Bass:
    if "nc" not in _CACHE:
        _CACHE["nc"] = _build()
    return _CACHE["nc"]


def run(inputs: dict, trace: bool = False):
    """Run on the 8 cores; returns (full_output, BassKernelResults)."""
    in_maps = _prep_in_maps(inputs)
    nc = _get_nc()
    kw = {}
    if trace:
        kw = dict(trace=True, trace_cores=list(range(NCORES)), stitch_traces=False)
    res = run_bass_kernel_spmd(nc, in_maps, core_ids=list(range(NCORES)), **kw)
    out = np.concatenate(
        [res.results[c]["out"] for c in range(NCORES)], axis=1
    ).astype(np.float32)
    return out, res


def kernel(**inputs) -> np.ndarray:
    out, _ = run(inputs)
    return out


# revision 26
# speedup vs baseline: 1.1739x; 1.0586x over previous
"""Trainium2 Bass kernel for nn_ODEFunc (gnn_message_passing, 8 cores).

Strategy (cost model: matmul = out-free-rows; DMA = free-dim bytes per
queue, 3 queues; collective = 15us + gathered bytes / 40GBps):
  - Batch-parallel branches: core b computes batch b's diff+adv gconv
    branches. All Chebyshev mats are built NODE-major with the support
    as the matmul *stationary* ([128,128] S^T blocks), so each x1/x2
    costs only 16 matmuls x fin rows instead of streaming the 512-wide
    support as moving data.
  - Node-major mats are packed per layer into nm stacks [128, J, 4, fin]
    (mat index J on the free dim -> no partition-start issues). The
    layer GEMM needs feature-major stationaries: batched PE transposes
    flip 8 (L1) / 2 (L2) mats per 128-row k-tile in one psum bank.
  - Layer GEMMs run with the (host-permuted) weights as *moving* data:
    out = c1/grads node-major, 64/16 rows per matmul. Biases are added
    with a ones-row rank-1 matmul into the same psum group.
  - W_d2/b_d2, W_a2/b_a2 are host-negated so tanh emits the grad sign;
    the diff 0.1 coefficient is one DVE op. Grads come out node-major,
    exactly the AllGather staging layout (no grad transposes).
  - AllGather [2,128,4,16] fp16 per core; gathered grads land straight
    in the W_f stationary gt_all[128, 40, KT] via two 3-dim strided
    DMAs (kt = m*16+f; wt is host-permuted to the same k enumeration).
    psX[40, 512]: X_diff rows 0-7, X_adv rows 32-39 (cols 8-31 are
    memset-zero lanes).
  - W_f shard (fp16, 16 MB) is split across the 3 DMA queues sized to
    each queue's idle windows (ACT's chunk is scheduled into the
    collective window via tile_wait_until).
  - GEMM half 1 finishes before half 2 starts so its gated-fusion chain
    hides under half 2's matmuls.
"""

import sys

sys.path.insert(0, "/opt/trn_rl_repo")

import numpy as np

import concourse.bass as bass
import concourse.mybir as mybir
from concourse import masks
from concourse.bass_utils import run_bass_kernel_spmd
from concourse.tile import TileContext
from concourse.vector_clock import ScopedClock

N = 512          # nodes
FL = 16          # latent
U = 64           # units
B = 8            # batch
HID = N * FL     # 8192
COEFF = 0.1
NCORES = 8
JS = HID // NCORES  # 1024 output columns per core
KT = HID // 128     # 64 contraction tiles for the W_f GEMM

f16 = mybir.dt.float16
f32 = mybir.dt.float32
AF = mybir.ActivationFunctionType
ALU = mybir.AluOpType

# smalls_f16 packed free-dim offsets (elements)
_OFF_X0M = 0             # [128, 4*16] x0 node-major
_OFF_WA1 = 64            # 3 tiles [128, 64]
_OFF_WD1 = 256           # [48(->128), 64]
_OFF_WA2 = 320           # 9 tiles [128, 16]
_OFF_WD2 = 464           # 2 tiles [128, 16]
_OFF_BF = 496            # [1, 1024]
_OFF_B1A = 1520          # [1, 64]
_OFF_B1D = 1584          # [1, 64]
_OFF_B2A = 1648          # [1, 16]
_OFF_B2D = 1664          # [1, 16]
_OFF_ONES = 1680         # [1, 128] ones
_SM16 = 1808


class PatchedTileContext(TileContext):
    """Tail drain with at most one sem wait per instruction.

    The walrus build here rejects Drain instructions carrying >2 sync
    waits ("Too many sync wait commands"). Spread the global-clock waits
    over individual SP nops ahead of the drain.
    """

    def _drain_and_barrier(self, tick_clock, wait_clock):
        nc = self.nc
        probe = nc.sync.nop(nofuse=True)
        wait_clock.add_sem_waits(
            probe.ins, ScopedClock({None: tick_clock.global_clock})
        )
        si = probe.ins.sync_info
        ws = list(si.on_wait) if si is not None else []
        if len(ws) > 1:
            probe.ins.sync_info = mybir.SyncInfo(
                on_wait=ws[:1], on_update=list(si.on_update)
            )
            for w in ws[1:]:
                n2 = nc.sync.nop(nofuse=True)
                n2.ins.sync_info = mybir.SyncInfo(on_wait=[w], on_update=[])
        nc.sync.drain()
        nc.all_engine_barrier()
        popped = nc._tile_sem_poison_stack.pop()
        assert popped is self._sem_poison
        nc.clear_and_free_semaphores(list(self.sems.allocated().values()))
        nc.all_engine_barrier()


_WAIT_LIMIT = 1


def _split_excess_waits(nc: bass.Bass) -> None:
    """Move sync waits beyond _WAIT_LIMIT onto same-engine NOPs inserted
    just before the carrying instruction (this walrus build has tiny
    setupSyncWait budgets for DMA/collective/drain instruction formats)."""
    for fn in nc.m.functions:
        for bb in fn.blocks:
            insts = bb.instructions
            i = 0
            while i < len(insts):
                inst = insts[i]
                si = inst.sync_info
                ws = list(si.on_wait) if si is not None and si.on_wait else []
                if len(ws) > _WAIT_LIMIT and type(inst).__name__ != "InstNoOp":
                    keep = ws[:_WAIT_LIMIT]
                    extra = ws[_WAIT_LIMIT:]
                    inst.sync_info = mybir.SyncInfo(
                        on_wait=keep, on_update=list(si.on_update)
                    )
                    for k, w in enumerate(extra):
                        nop = mybir.InstNoOp(
                            name=f"{inst.name}-w{k}",
                            engine=inst.engine,
                            bass_nofuse=True,
                            sync_info=mybir.SyncInfo(on_wait=[w], on_update=[]),
                        )
                        nc.register_instruction(nop, overwrite=True)
                        insts.insert(i, nop)
                        i += 1
                i += 1


def _build(collective: bool = True) -> bass.Bass:
    nc = bass.Bass(num_devices=NCORES)

    # ---- DRAM I/O (per-core values supplied via in_maps) ----
    sm16_d = nc.dram_tensor("sm16", [128, _SM16], f16, kind="ExternalInput")
    sup_d = nc.dram_tensor("supT", [3, 128, 3, 4, N], f16, kind="ExternalInput")
    wt_d = nc.dram_tensor("wt", [128, KT, JS], f16, kind="ExternalInput")
    out_d = nc.dram_tensor("out", [B, JS], f32, kind="ExternalOutput")

    with PatchedTileContext(nc) as tc:
        from contextlib import ExitStack

        with ExitStack() as ctx:
            const_p = ctx.enter_context(tc.tile_pool(name="const", bufs=1))
            sup_p = ctx.enter_context(tc.tile_pool(name="sup", bufs=1))
            sc_p = ctx.enter_context(tc.tile_pool(name="sc", bufs=1))
            fus_p = ctx.enter_context(tc.tile_pool(name="fus", bufs=1))
            fu_p = ctx.enter_context(tc.tile_pool(name="fu", bufs=5))
            acc_p = ctx.enter_context(tc.tile_pool(name="acc", bufs=3, space="PSUM"))
            psx_p = ctx.enter_context(tc.tile_pool(name="psx", bufs=2, space="PSUM"))
            tr_p = ctx.enter_context(tc.tile_pool(name="tr", bufs=2, space="PSUM"))
            dram_p = ctx.enter_context(tc.tile_pool(name="dram", bufs=1, space="DRAM"))

            # ---- constants / memsets ----
            id128 = const_p.tile([128, 128], f16, tag="id")
            masks.make_identity(nc, id128[:])
            ones40 = const_p.tile([1, 40], f16, tag="ones")
            nc.vector.memset(ones40[:], 1.0)
            # W_f-GEMM stationary [q, col, kt]; cols 8-31 are never DMA'd
            # -> zero them once, early (garbage would NaN the psum)
            gt_all = const_p.tile([128, 40, KT], f16, tag="gt")
            nc.gpsimd.memset(gt_all[:], 0.0)

            # ---- input DMAs: smalls first; supports split SP/Pool so the
            # last support lands by ~9.5us ----
            sm16 = const_p.tile([128, _SM16], f16, tag="sm16")
            nc.sync.dma_start(sm16[:], sm16_d[:])
            sup_tiles = []
            for s in range(9):
                supb = sup_p.tile([128, 4, N], f16, tag=f"sup{s}")
                sup_tiles.append(supb)
            for s in (0, 2, 4, 6, 8):
                nc.sync.dma_start(sup_tiles[s][:], sup_d[s // 3][:, s % 3])
            for s in (1, 3, 5, 7):
                nc.gpsimd.dma_start(sup_tiles[s][:], sup_d[s // 3][:, s % 3])

            # W_f shard: SP streams most of it through the branch phase;
            # ACT's chunk is scheduled into the collective window. Pool
            # carries none (it does late-branch copies + the collective).
            wt_all = const_p.tile([128, KT, JS], f16, tag="wt")
            nc.sync.dma_start(wt_all[:, 0:40, :], wt_d[:, 0:40, :])
            with tc.tile_wait_until(0.024):
                nc.scalar.dma_start(wt_all[:, 40:64, :], wt_d[:, 40:64, :])

            # packed-small views
            x0m_all = sm16[:, _OFF_X0M : _OFF_X0M + 64]

            def x0m_ap(m):
                return sm16[:, _OFF_X0M + m * FL : _OFF_X0M + (m + 1) * FL]

            def wa1_ap(t, k=128):
                return sm16[0:k, _OFF_WA1 + t * U : _OFF_WA1 + (t + 1) * U]

            wd1_ap = sm16[0:48, _OFF_WD1 : _OFF_WD1 + U]

            def wa2_ap(t, k=128):
                return sm16[0:k, _OFF_WA2 + t * FL : _OFF_WA2 + (t + 1) * FL]

            def wd2_ap(t, k=128):
                return sm16[0:k, _OFF_WD2 + t * FL : _OFF_WD2 + (t + 1) * FL]

            def bf_ap(lo, hi):
                return sm16[0:1, _OFF_BF + lo : _OFF_BF + hi]

            b1a = sm16[0:1, _OFF_B1A : _OFF_B1A + U]
            b1d = sm16[0:1, _OFF_B1D : _OFF_B1D + U]
            b2a = sm16[0:1, _OFF_B2A : _OFF_B2A + FL]
            b2d = sm16[0:1, _OFF_B2D : _OFF_B2D + FL]
            ones128 = sm16[0:1, _OFF_ONES : _OFF_ONES + 128]

            # node-major mat stacks [128, J, m, fin]; J: 0=x0/c1, then
            # x1_s at 1+2s, x2_s at 2+2s (the reference concat order)
            nm1a = const_p.tile([128, 4, 17, FL], f16, tag="nm1a")
            nm1d = const_p.tile([128, 4, 3, FL], f16, tag="nm1d")
            nm2a = const_p.tile([128, 4, 17, U], f16, tag="nm2a")
            nm2d = const_p.tile([128, 4, 3, U], f16, tag="nm2d")

            nc.vector.tensor_copy(nm1a[:, :, 0, :], x0m_all)
            nc.scalar.copy(nm1d[:, :, 0, :], x0m_all)

            def supT_ap(s, m, j):
                # S_s^T[m-block, j-block] = stationary for out node-block j
                return sup_tiles[s][:, m, j * 128 : (j + 1) * 128]

            # copy/stt helpers: 0 = DVE, 1 = ACT (copy only), 2 = Pool
            def cp(which, dst, src):
                if which == 1:
                    nc.scalar.copy(dst, src)
                elif which == 2:
                    nc.gpsimd.tensor_copy(dst, src)
                else:
                    nc.vector.tensor_copy(dst, src)

            def cheb_nm(fin, nm, s_list, x_src, pool_tag, cp_rot):
                """x1 = S@x, x2 = 2*S@x1 - x in node-major form.

                x_src(s): [128, 4, fin] node-major input (x0 or c1).
                Writes nm[:, 1+2i, :, :] and nm[:, 2+2i, :, :] for s_list[i].
                """
                # pass A: x1 for every support
                for i, s in enumerate(s_list):
                    psb = acc_p.tile([128, 4, U], f32, tag="ps")
                    ps1 = psb[:, :, 0:fin]
                    for j in range(4):
                        for m in range(4):
                            nc.tensor.matmul(
                                ps1[:, j, :], supT_ap(s, m, j),
                                x_src(s)[:, m, :],
                                start=(m == 0), stop=(m == 3),
                                skip_group_check=True,
                            )
                    cp(cp_rot[i % len(cp_rot)], nm[s][:, :, 1 + 2 * i, :], ps1[:])
                # pass B: x2 = 2*(S@x1) - x
                for i, s in enumerate(s_list):
                    psb = acc_p.tile([128, 4, U], f32, tag="ps")
                    ps2 = psb[:, :, 0:fin]
                    for j in range(4):
                        for m in range(4):
                            nc.tensor.matmul(
                                ps2[:, j, :], supT_ap(s, m, j),
                                nm[s][:, m, 1 + 2 * i, :],
                                start=(m == 0), stop=(m == 3),
                                skip_group_check=True,
                            )
                    cp(cp_rot[(i + 1) % len(cp_rot)],
                       nm[s][:, :, 2 + 2 * i, :], ps2[:])

            def fm_transpose(nm_ap_fn, rows, tag, eng):
                """Batched FM flip: nm cols (J..J+g, fin) of each m-block ->
                fm tile [rows<=128, 512] (k = J*fin+f, node-major cols)."""
                fm = sc_p.tile([rows, N], f16, tag=tag)
                ptr = tr_p.tile([128, N], f16, tag="ptr")
                for m in range(4):
                    nc.tensor.transpose(
                        ptr[0:rows, m * 128 : (m + 1) * 128],
                        nm_ap_fn(m),
                        id128[:],
                    )
                cp(eng, fm[:], ptr[0:rows, :])
                return fm

            # ---- Layer 1 (fin=16) ----
            adv = list(range(8))
            cheb_nm(FL, {s: nm1a for s in adv}, adv,
                    lambda s: nm1a[:, :, 0, :], "psL1", (0, 1))
            cheb_nm(FL, {8: nm1d}, [8],
                    lambda s: nm1d[:, :, 0, :], "psL1", (1, 0))

            # FM stationaries for the L1 GEMM: adv tiles (8+8+1 mats),
            # diff tile (3 mats)
            fm1a = []
            for t in range(2):
                fm1a.append(fm_transpose(
                    lambda m, t=t: nm1a[:, m, 8 * t : 8 * t + 8, :],
                    128, f"fm1a{t}", t % 2,
                ))
            fm1a.append(fm_transpose(
                lambda m: nm1a[:, m, 16, :], FL, "fm1a2", 0))
            fm1d = fm_transpose(
                lambda m: nm1d[:, m, 0:3, :], 48, "fm1d", 1)

            # L1 GEMM (weights moving): c1 = tanh(xs @ W1 + b1), node-major
            pc1a = acc_p.tile([128, 4, U], f32, tag="ps")
            pc1d = acc_p.tile([128, 4, U], f32, tag="ps")
            for j in range(4):
                for t in range(3):
                    kk = 128 if t < 2 else FL
                    nc.tensor.matmul(
                        pc1a[:, j, :], fm1a[t][0:kk, j * 128 : (j + 1) * 128],
                        wa1_ap(t, kk),
                        start=(t == 0), stop=False, skip_group_check=True,
                    )
                nc.tensor.matmul(
                    pc1a[:, j, :], ones128, b1a,
                    start=False, stop=True, skip_group_check=True,
                )
                nc.tensor.matmul(
                    pc1d[:, j, :], fm1d[:, j * 128 : (j + 1) * 128], wd1_ap,
                    start=True, stop=False, skip_group_check=True,
                )
                nc.tensor.matmul(
                    pc1d[:, j, :], ones128, b1d,
                    start=False, stop=True, skip_group_check=True,
                )
            nc.scalar.activation(nm2a[:, :, 0, :], pc1a[:], AF.Tanh)
            nc.scalar.activation(nm2d[:, :, 0, :], pc1d[:], AF.Tanh)

            # ---- Layer 2 (fin=64); diff first so its grad chain overlaps ----
            cheb_nm(U, {8: nm2d}, [8],
                    lambda s: nm2d[:, :, 0, :], "psL2", (1, 0))
            cheb_nm(U, {s: nm2a for s in adv}, adv,
                    lambda s: nm2a[:, :, 0, :], "psL2", (0, 1))

            # FM stationaries for the L2 GEMM: diff (2 tiles), adv (9 tiles)
            fm2d = []
            fm2d.append(fm_transpose(
                lambda m: nm2d[:, m, 0:2, :], 128, "fm2d0", 0))
            fm2d.append(fm_transpose(
                lambda m: nm2d[:, m, 2, :], U, "fm2d1", 1))
            fm2a = []
            for t in range(8):
                fm2a.append(fm_transpose(
                    lambda m, t=t: nm2a[:, m, 2 * t : 2 * t + 2, :],
                    128, f"fm2a{t}", (0, 1)[t % 2],
                ))
            fm2a.append(fm_transpose(
                lambda m: nm2a[:, m, 16, :], U, "fm2a8", 1))

            # L2 GEMMs -> grads node-major [128, 4, 16] (the agin layout).
            # W2/b2 are host-negated so tanh lands the sign; diff still
            # needs the 0.1 coefficient.
            g_st = fus_p.tile([128, 2, 4, FL], f16, tag="gst")
            pgdb = acc_p.tile([128, 4, U], f32, tag="ps")
            pgd = pgdb[:, :, 0:FL]
            for j in range(4):
                for t in range(2):
                    kk = 128 if t < 1 else U
                    nc.tensor.matmul(
                        pgd[:, j, :], fm2d[t][0:kk, j * 128 : (j + 1) * 128],
                        wd2_ap(t, kk),
                        start=(t == 0), stop=False, skip_group_check=True,
                    )
                nc.tensor.matmul(
                    pgd[:, j, :], ones128, b2d,
                    start=False, stop=True, skip_group_check=True,
                )
            gd_t = sc_p.tile([128, 4, FL], f16, tag="gdt")
            nc.scalar.activation(gd_t[:], pgd[:], AF.Tanh)
            nc.vector.tensor_scalar_mul(g_st[:, 0, :, :], gd_t[:], COEFF)

            pgab = acc_p.tile([128, 4, U], f32, tag="ps")
            pga = pgab[:, :, 0:FL]
            for j in range(4):
                for t in range(9):
                    kk = 128 if t < 8 else U
                    nc.tensor.matmul(
                        pga[:, j, :], fm2a[t][0:kk, j * 128 : (j + 1) * 128],
                        wa2_ap(t, kk),
                        start=(t == 0), stop=False, skip_group_check=True,
                    )
                nc.tensor.matmul(
                    pga[:, j, :], ones128, b2a,
                    start=False, stop=True, skip_group_check=True,
                )
            nc.scalar.activation(g_st[:, 1, :, :], pga[:], AF.Tanh)

            # ---- AllGather of node-major grads: agin[r, p, m, f] ----
            agin = dram_p.tile([2, 128, 4, FL], f16)
            agout = dram_p.tile([NCORES, 2, 128, 4, FL], f16)
            nc.sync.dma_start(agin[0].rearrange("p m f -> p (m f)"),
                              g_st[:, 0].rearrange("p m f -> p (m f)"))
            nc.scalar.dma_start(agin[1].rearrange("p m f -> p (m f)"),
                                g_st[:, 1].rearrange("p m f -> p (m f)"))
            if collective:
                nc.gpsimd.collective_compute(
                    "AllGather",
                    ALU.bypass,
                    replica_groups=[list(range(NCORES))],
                    ins=[agin.opt()],
                    outs=[agout.opt()],
                )
            else:
                for r in range(NCORES):
                    nc.gpsimd.dma_start(agout[r], agin[:])

            # ---- W_f phase ----
            # Gathered grads land directly in the stationary layout:
            # gt_all[q, col, kt] with kt = m*16+f <-> k = (m*128+q)*16+f;
            # wt is host-permuted to the same enumeration. Diff grads ->
            # cols 0-7, adv -> cols 32-39 (3-dim APs, contiguous last dim).
            nc.sync.dma_start(
                gt_all[:, 0:8, :],
                agout[:, 0].rearrange("c p m f -> p c (m f)"),
            )
            nc.scalar.dma_start(
                gt_all[:, 32:40, :],
                agout[:, 1].rearrange("c p m f -> p c (m f)"),
            )

            # Half 1's GEMM completes first so its fusion chain runs under
            # half 2's GEMM; each half is 64 matmuls + a bias row.
            def fusion(ps, h):
                # only one PSUM operand allowed per DVE op -> stage X_adv
                xa = fu_p.tile([B, 512], f16, tag="fu")
                nc.scalar.copy(xa[:], ps[32 : 32 + B, :])
                ssum = fu_p.tile([B, 512], f16, tag="fu")
                nc.vector.tensor_add(ssum[:], ps[0:B, :], xa[:])
                d = fu_p.tile([B, 512], f16, tag="fu")
                nc.vector.tensor_sub(d[:], ps[0:B, :], xa[:])
                z = fu_p.tile([B, 512], f16, tag="fu")
                nc.scalar.activation(z[:], ssum[:], AF.Sigmoid)
                zd = fu_p.tile([B, 512], f16, tag="fu")
                nc.vector.tensor_mul(zd[:], z[:], d[:])
                o = fus_p.tile([B, 512], f32, tag="fo")
                nc.vector.tensor_add(o[:], zd[:], ps[32 : 32 + B, :])
                nc.sync.dma_start(out_d[:, h * 512 : (h + 1) * 512], o[:])

            psX1 = psx_p.tile([40, 512], f32, tag="psX")
            psX2 = psx_p.tile([40, 512], f32, tag="psX")
            for kt in range(KT):
                nc.tensor.matmul(
                    psX1[:], gt_all[:, :, kt], wt_all[:, kt, 0:512],
                    start=(kt == 0), stop=False, skip_group_check=True,
                )
            nc.tensor.matmul(
                psX1[:], ones40[:], bf_ap(0, 512),
                start=False, stop=True, skip_group_check=True,
            )
            fusion(psX1, 0)
            for kt in range(KT):
                nc.tensor.matmul(
                    psX2[:], gt_all[:, :, kt], wt_all[:, kt, 512:JS],
                    start=(kt == 0), stop=False, skip_group_check=True,
                )
            nc.tensor.matmul(
                psX2[:], ones40[:], bf_ap(512, JS),
                start=False, stop=True, skip_group_check=True,
            )
            fusion(psX2, 1)

    _split_excess_waits(nc)
    return nc


def _prep_in_maps(inputs: dict) -> list[dict]:
    y = np.asarray(inputs["y"], np.float32)
    sd = np.asarray(inputs["supports_diff"], np.float32)
    sa = np.asarray(inputs["supports_adv"], np.float32)
    W_d1 = np.asarray(inputs["W_d1"], np.float32)
    W_d2 = -np.asarray(inputs["W_d2"], np.float32)
    W_a1 = np.asarray(inputs["W_a1"], np.float32)
    W_a2 = -np.asarray(inputs["W_a2"], np.float32)
    W_f = np.asarray(inputs["W_f"], np.float32)
    b_f = np.asarray(inputs["b_f"], np.float32)


    def cheb_fold(W, fin, M):
        # mats become [x0, x1_s, y2_s=S@x1_s]: W'[x0] -= sum W[x2_s];
        # W'[y2_s] = 2 W[x2_s]
        Wf = W.reshape(fin, M, -1).copy()
        for j in range(2, M, 2):
            Wf[:, 0, :] -= Wf[:, j, :]
            Wf[:, j, :] *= 2.0
        return Wf.reshape(fin * M, -1)

    W_a1 = cheb_fold(W_a1, FL, 17)
    W_d1 = cheb_fold(W_d1, FL, 3)
    W_a2 = cheb_fold(W_a2, U, 17)
    W_d2 = cheb_fold(W_d2, U, 3)

    # supports, transposed, node-tile-major, one per tile:
    # supT[b, p, si, m, n] = S_{3b+si}.T[m*128+p, n]
    supT = np.empty((3, 128, 3, 4, N), np.float16)
    for s in range(9):
        Ssrc = sa[s] if s < 8 else sd[0]
        st = Ssrc.T.astype(np.float16)  # [m, n]
        supT[s // 3, :, s % 3] = st.reshape(4, 128, N).transpose(1, 0, 2)

    def perm_pad(W, fin, M, fout, ntiles):
        # reference row (f, m) -> packed row m*fin+f, zero-padded to tiles
        Wp = W.reshape(fin, M, fout).transpose(1, 0, 2).reshape(fin * M, fout)
        pad = np.zeros((ntiles * 128, fout), np.float16)
        pad[: fin * M] = Wp.astype(np.float16)
        return pad.reshape(ntiles, 128, fout)

    wa1 = perm_pad(W_a1, FL, 17, U, 3)
    wd1 = perm_pad(W_d1, FL, 3, U, 1)
    wa2 = perm_pad(W_a2, U, 17, FL, 9)
    wd2 = perm_pad(W_d2, U, 3, FL, 2)

    # wt[q, m*16+f, j] = W_f.T[(m*128+q)*FL + f, c*JS+j]  (kt = m*16+f)
    WT = W_f.T.astype(np.float16)  # [k_orig = n*FL+f, j_global]
    in_maps = []
    for c in range(NCORES):
        x0 = y[c].reshape(N, FL)  # [node, f]
        x0m = x0.reshape(4, 128, FL).transpose(1, 0, 2).astype(np.float16)

        sm16 = np.zeros((128, _SM16), np.float16)
        sm16[:, _OFF_X0M : _OFF_X0M + 64] = x0m.reshape(128, 64)
        sm16[:, _OFF_WA1 : _OFF_WA1 + 3 * U] = wa1.transpose(1, 0, 2).reshape(
            128, 3 * U
        )
        sm16[:, _OFF_WD1 : _OFF_WD1 + U] = wd1[0]
        sm16[:, _OFF_WA2 : _OFF_WA2 + 9 * FL] = wa2.transpose(1, 0, 2).reshape(
            128, 9 * FL
        )
        sm16[:, _OFF_WD2 : _OFF_WD2 + 2 * FL] = wd2.transpose(1, 0, 2).reshape(
            128, 2 * FL
        )
        sm16[0, _OFF_BF : _OFF_BF + JS] = b_f[c * JS : (c + 1) * JS].astype(
            np.float16
        )
        sm16[0, _OFF_B1A : _OFF_B1A + U] = np.asarray(inputs["b_a1"], np.float16)
        sm16[0, _OFF_B1D : _OFF_B1D + U] = np.asarray(inputs["b_d1"], np.float16)
        sm16[0, _OFF_B2A : _OFF_B2A + FL] = -np.asarray(
            inputs["b_a2"], np.float16
        )
        sm16[0, _OFF_B2D : _OFF_B2D + FL] = -np.asarray(
            inputs["b_d2"], np.float16
        )
        sm16[0, _OFF_ONES : _OFF_ONES + 128] = 1.0

        # [(m q f), j] -> [q, m, f, j] -> [q, m*16+f, j]
        wt = np.ascontiguousarray(
            WT[:, c * JS : (c + 1) * JS]
            .reshape(4, 128, FL, JS)
            .transpose(1, 0, 2, 3)
            .reshape(128, KT, JS)
        )
        in_maps.append({"sm16": sm16, "supT": supT, "wt": wt})
    return in_maps


_CACHE: dict = {}


def _get_nc() -> bass.Bass:
    if "nc" not in _CACHE:
        _CACHE["nc"] = _build()
    return _CACHE["nc"]


def run(inputs: dict, trace: bool = False):
    """Run on the 8 cores; returns (full_output, BassKernelResults)."""
    in_maps = _prep_in_maps(inputs)
    nc = _get_nc()
    kw = {}
    if trace:
        kw = dict(trace=True, trace_cores=list(range(NCORES)), stitch_traces=False)
    res = run_bass_kernel_spmd(nc, in_maps, core_ids=list(range(NCORES)), **kw)
    out = np.concatenate(
        [res.results[c]["out"] for c in range(NCORES)], axis=1
    ).astype(np.float32)
    return out, res


def kernel(**inputs) -> np.ndarray:
    out, _ = run(inputs)
    return out
